# revision 23
# baseline (speedup 1.0000x reference)
"""CopyGenerator kernel for 8 Trainium2 NeuronCores.

Strategy: tensor-parallel over the vocab dimension, collective-free.
  - Each core computes logits = hidden @ W[:, k*4000:(k+1)*4000] (bf16 matmul,
    fp32 accumulate) and applies exp via ACT with a per-row bias ln(1-p_copy),
    so the activation directly emits e = exp(logit)*(1-p_copy) in bf16,
    streamed to DRAM, with the fused row-sum (accum_out) kept as fp32
    partials.
  - No AllReduce: the softmax denominator is finished on the host - each
    core returns its [128, 2, 16] row-sum partials (16 KB) and the host
    sums them across cores and applies the 1/Z row scale while upcasting
    the bf16 shards to the fp32 output.
  - p_copy = sigmoid(hidden @ Wc + bc) is a [2048,512]x[512,1] matvec,
    computed on the host; the device receives ln(1-p_copy) as an ACT bias
    and a pre-scaled attention (attn * p_copy) for the copy path.
  - Copy path (einsum over src_map) sharded 4 batches per core on the PE,
    emitted before the main loop so it runs while W streams in.
Host side: shard/cast inputs, run SPMD on cores 0-7, normalize + gather.
"""

import numpy as np
import ml_dtypes

bf16 = ml_dtypes.bfloat16

# Problem shape (hardcoded per contract)
B, T, S, C, D, V = 32, 64, 400, 100, 512, 32000
R = B * T              # 2048 rows, row r = t*32 + b
NC = 8
VS = V // NC           # 4000 vocab cols per core
PAD_IDX = 1
NEG_INF = -1e9

KCH = D // 128         # 4 contraction chunks of 128
NRB = R // 128         # 16 row blocks
SCH = 4                # s-chunks of 100 for the copy einsum
OUT_BUFS = 3

_cache = {}


def _build(all_bias: bool):
    import concourse.bass as bass
    import concourse.mybir as mybir
    import concourse.tile as tile
    from concourse import bacc

    fp32 = mybir.dt.float32
    bf = mybir.dt.bfloat16
    AF = mybir.ActivationFunctionType

    nc = bacc.Bacc("TRN2", target_bir_lowering=False, debug=False, num_devices=NC)

    # ---- I/O ----
    hT_d = nc.dram_tensor("hT", [D, R], bf, kind="ExternalInput")
    W_d = nc.dram_tensor("Wk", [D, VS], bf, kind="ExternalInput")
    lnb_d = nc.dram_tensor("lnb", [128, NRB], fp32, kind="ExternalInput")
    attnT_d = nc.dram_tensor("attnT", [S, 256], bf, kind="ExternalInput")
    srcmap_d = nc.dram_tensor("srcmap", [S, 4 * C], bf, kind="ExternalInput")
    out_d = nc.dram_tensor("out", [R, VS], bf, kind="ExternalOutput")
    rs_d = nc.dram_tensor("rs", [128, 8 * NRB], fp32, kind="ExternalOutput")
    cp_d = nc.dram_tensor("cp", [T, 4 * C], fp32, kind="ExternalOutput")
    if all_bias:
        bias_d = nc.dram_tensor("biask", [1, VS], bf, kind="ExternalInput")

    with tile.TileContext(nc) as tc:
        with (
            tc.tile_pool(name="sb", bufs=1) as sb,
            tc.tile_pool(name="ps", bufs=4, space="PSUM") as ps,
        ):
            # ---- resident loads ----
            # sync (HWDGE) ring leads with the eight 0.5MB W column-chunks
            # that gate the matmul stream; the bulk of hT rides at its tail.
            # The gpsimd SWDGE ring carries, in parallel: lnb, the first hT
            # row-chunks (which gate the first stripes), and the copy-path
            # inputs.
            hT_sb = sb.tile([128, KCH, R], bf)
            hT_view = hT_d.ap().rearrange("(c p) r -> p c r", p=128)
            W_sb = sb.tile([128, KCH, VS], bf)
            W_view = W_d.ap().rearrange("(c p) v -> p c v", p=128)
            # sync ring: hT head + W chunks 1-7; gpsimd ring (parallel):
            # W chunk 0, lnb, hT row-chunk 1, copy-path inputs, hT bulk.
            nc.sync.dma_start(hT_sb[:, :, 0:384], hT_view[:, :, 0:384])
            for q in range(1, 8):
                nc.sync.dma_start(W_sb[:, :, q * 500:(q + 1) * 500],
                                  W_view[:, :, q * 500:(q + 1) * 500])

            nc.gpsimd.dma_start(W_sb[:, :, 0:500], W_view[:, :, 0:500])
            lnb_sb = sb.tile([128, NRB], fp32)
            nc.gpsimd.dma_start(lnb_sb[:, :], lnb_d.ap())
            nc.gpsimd.dma_start(hT_sb[:, :, 384:512], hT_view[:, :, 384:512])
            attnT_sb = sb.tile([100, SCH, 256], bf)
            nc.gpsimd.dma_start(attnT_sb[:, :, :], attnT_d.ap().rearrange("(c p) j -> p c j", p=100))
            srcmap_sb = sb.tile([100, SCH, 4 * C], bf)
            nc.gpsimd.dma_start(srcmap_sb[:, :, :], srcmap_d.ap().rearrange("(c p) j -> p c j", p=100))
            for rq in range(1, 4):
                nc.gpsimd.dma_start(hT_sb[:, :, rq * 512:(rq + 1) * 512],
                                    hT_view[:, :, rq * 512:(rq + 1) * 512])
            if all_bias:
                bias_sb = sb.tile([1, VS], bf)
                nc.gpsimd.dma_start(bias_sb[:, :], bias_d.ap())
                ones_sb = sb.tile([1, 128], bf)
                nc.vector.memset(ones_sb[:, :], 1.0)

            rs_sb = sb.tile([128, 8 * NRB], fp32)  # rowsum partials [p, rb*8+c]
            nc.vector.memset(rs_sb[:, :], 0.0)
            cp_sb = sb.tile([64, 4 * C], fp32)

            ot_tiles = {}

            def get_ot(rb):
                if rb not in ot_tiles:
                    ot_tiles[rb] = sb.tile([128, VS], bf, tag="ot",
                                           bufs=OUT_BUFS, name=f"ot{rb}")
                return ot_tiles[rb]

            def stripe(rb, c0, nb):
                """One nb*500-col stripe: matmuls + exp with fused bias/accum."""
                ot = get_ot(rb)
                st = ps.tile([128, 2, 512], fp32, tag="stripe",
                             name=f"l{rb}_{c0}")
                for kk in range(KCH):
                    for j in range(nb):
                        nc.tensor.matmul(
                            st[:, j, 0:500],
                            hT_sb[:, kk, rb * 128:(rb + 1) * 128],
                            W_sb[:, kk, (c0 + j) * 500:(c0 + j + 1) * 500],
                            start=(kk == 0),
                            stop=(kk == KCH - 1 and not all_bias))
                if all_bias:
                    for j in range(nb):
                        nc.tensor.matmul(
                            st[:, j, 0:500], ones_sb[:, :],
                            bias_sb[:, (c0 + j) * 500:(c0 + j + 1) * 500],
                            start=False, stop=True)
                ev = ot[:, c0 * 500:(c0 + nb) * 500]
                if nb > 1:
                    ev = ev.rearrange("p (g v) -> p g v", g=nb)
                    si = st[:, :, 0:500]
                else:
                    si = st[:, 0, 0:500]
                nc.scalar.activation(
                    ev, si, AF.Exp,
                    bias=lnb_sb[:, rb:rb + 1],
                    accum_out=rs_sb[:, rb * 8 + c0:rb * 8 + c0 + 1])

            def emit_out(rb, c0=0, c1=8):
                nc.sync.dma_start(
                    out_d.ap()[rb * 128:(rb + 1) * 128, c0 * 500:c1 * 500],
                    ot_tiles[rb][:, c0 * 500:c1 * 500])

            # ---- PE warmup: ~3.4us of dummy matmuls on zero tiles so the
            # HAM clock gate is at 8/8 (2.4 GHz) when the real stream
            # starts; they also fill the input-DMA wait ----
            wu_w = sb.tile([128, 128], bf)
            wu_x = sb.tile([128, 512], bf)
            nc.vector.memset(wu_w[:, :], 0.0)
            nc.vector.memset(wu_x[:, :], 0.0)
            wu_ps = ps.tile([128, 2, 512], fp32, tag="stripe", name="warm")
            for i in range(8):
                nc.tensor.matmul(wu_ps[:, 0, :], wu_w[:, :], wu_x[:, :],
                                 start=True, stop=True)

            # ---- phase 1: chunk-major over rb0-2 with 500-col stripes so
            # the PE starts as soon as the first 0.5MB W chunk lands ----
            NW = 3
            for q in range(8):
                for rb in range(NW):
                    stripe(rb, q, 1)
                if q == 3:
                    # copy path: cp[t, bb*C:(bb+1)*C] =
                    #   sum_s attnT[s, bb*64+t] * srcmap[s, bb, :]
                    # (attnT pre-scaled by p_copy on the host)
                    cpps = ps.tile([64, 4 * C], fp32, tag="stripe", name="cpps")
                    for bb in range(4):
                        for c in range(SCH):
                            nc.tensor.matmul(
                                cpps[:, bb * C:(bb + 1) * C],
                                attnT_sb[:, c, bb * 64:(bb + 1) * 64],
                                srcmap_sb[:, c, bb * C:(bb + 1) * C],
                                start=(c == 0), stop=(c == SCH - 1))
                    nc.vector.tensor_copy(cp_sb[:, :], cpps[:, :])
                    nc.gpsimd.dma_start(cp_d.ap(), cp_sb[:, :])
            for rb in range(NW):
                emit_out(rb)

            # ---- phase 2: row-major for the rest, 1000-col stripes; the
            # last row block streams its output in two halves so the final
            # DMA is small ----
            for rb in range(NW, NRB):
                for q in range(4):
                    stripe(rb, 2 * q, 2)
                    if rb == NRB - 1 and q >= 1:
                        # stream the last row block out in shrinking pieces
                        # so the final DMA is small
                        emit_out(rb, 2 * (q - 1) if q == 1 else 2 * q,
                                 2 * q + 2)
                if rb == NRB - 1:
                    # all rowsum slots except rb15's are final now
                    nc.gpsimd.dma_start(rs_d.ap()[:, 0:8 * (NRB - 1)],
                                        rs_sb[:, 0:8 * (NRB - 1)])
                else:
                    emit_out(rb)

            nc.sync.dma_start(rs_d.ap()[:, 8 * (NRB - 1):],
                              rs_sb[:, 8 * (NRB - 1):])

    nc.compile()
    return nc


def _get_nc(all_bias: bool):
    key = ("nc", all_bias)
    if key not in _cache:
        _cache[key] = _build(all_bias)
    return _cache[key]


def kernel(hidden, attn, src_map, W, b, Wc, bc):
    from concourse.bass_utils import run_bass_kernel_spmd

    hidden = np.asarray(hidden, dtype=np.float32)
    attn = np.asarray(attn, dtype=np.float32)
    src_map = np.asarray(src_map, dtype=np.float32)
    W = np.asarray(W, dtype=np.float32)
    b = np.asarray(b, dtype=np.float32)
    Wc = np.asarray(Wc, dtype=np.float32)
    bc = np.asarray(bc, dtype=np.float32)

    all_bias = bool(np.any(b != 0.0))

    # host prologue: p_copy (tiny matvec) and the per-row ACT bias ln(1-p)
    z = hidden.astype(np.float64) @ Wc.astype(np.float64) + bc.astype(np.float64)
    p = 1.0 / (1.0 + np.exp(-z))                         # [R, 1]
    one_m_p = (1.0 - p).reshape(-1)                      # [R]
    lnb = np.log(one_m_p).reshape(NRB, 128).T.astype(np.float32)  # [128, NRB]
    lnb = np.ascontiguousarray(lnb)

    hT = np.ascontiguousarray(hidden.T).astype(bf16)     # [512, 2048]
    attnS = attn * p.astype(np.float32)                  # [R, S] attn * p_copy

    nc = _get_nc(all_bias)

    in_maps = []
    for k in range(NC):
        Wk = np.ascontiguousarray(W[:, k * VS:(k + 1) * VS]).astype(bf16)

        # copy-path shard: batches 4k..4k+3, packed col j = bb*64 + t
        rows = np.array([[t * 32 + 4 * k + bb for t in range(T)] for bb in range(4)])
        rows_flat = rows.reshape(-1)
        attnT_k = np.ascontiguousarray(attnS[rows_flat, :].T).astype(bf16)   # [400, 256]
        srcmap_k = np.ascontiguousarray(
            src_map[:, 4 * k:4 * k + 4, :].reshape(S, 4 * C)).astype(bf16)  # [400, 400]

        im = {"hT": hT, "Wk": Wk, "lnb": lnb, "attnT": attnT_k, "srcmap": srcmap_k}
        if all_bias:
            bias_k = b[k * VS:(k + 1) * VS].astype(np.float64)
            if k == 0:
                bias_k = bias_k.copy()
                bias_k[PAD_IDX] += NEG_INF
            im["biask"] = bias_k.astype(bf16)[None, :]                      # [1, 4000]
        in_maps.append(im)

    global _last_in_maps
    _last_in_maps = in_maps
    res = run_bass_kernel_spmd(nc, in_maps, core_ids=list(range(NC))).results

    # host epilogue: finish the softmax denominator and normalize while
    # upcasting the bf16 shards.
    rs_tot = np.zeros((128, NRB), dtype=np.float64)
    for k in range(NC):
        rsk = res[k]["rs"].astype(np.float64).reshape(128, NRB, 8)
        rs_tot += rsk.sum(axis=2)
    zp = rs_tot.T.reshape(-1)                            # [R] = (1-p) * (Z + e_pad)

    full = np.empty((R, V + C), dtype=np.float32)
    for k in range(NC):
        full[:, k * VS:(k + 1) * VS] = res[k]["out"]

    if all_bias:
        # PAD handled via the -1e9 bias on the device (exp underflows to 0)
        zrow = zp / one_m_p                              # Z_true
    else:
        # device computed exp(0)=1 at the PAD column; remove it from Z
        e_pad = full[:, PAD_IDX].astype(np.float64) / one_m_p
        zrow = zp / one_m_p - e_pad
    scale = (1.0 / zrow).astype(np.float32)
    full[:, :V] *= scale[:, None]
    full[:, PAD_IDX] = 0.0

    t_idx = np.arange(T) * 32
    for k in range(NC):
        cp = res[k]["cp"].reshape(T, 4, C)
        for bb in range(4):
            full[t_idx + 4 * k + bb, V:] = cp[:, bb, :]
    return full


# revision 26
# speedup vs baseline: 1.0008x; 1.0008x over previous
"""CopyGenerator kernel for 8 Trainium2 NeuronCores.

Strategy: tensor-parallel over the vocab dimension, collective-free.
  - Each core computes logits = hidden @ W[:, k*4000:(k+1)*4000] (bf16 matmul,
    fp32 accumulate) and applies exp via ACT with a per-row bias ln(1-p_copy),
    so the activation directly emits e = exp(logit)*(1-p_copy) in bf16,
    streamed to DRAM, with the fused row-sum (accum_out) kept as fp32
    partials.
  - No AllReduce: the softmax denominator is finished on the host - each
    core returns its [128, 2, 16] row-sum partials (16 KB) and the host
    sums them across cores and applies the 1/Z row scale while upcasting
    the bf16 shards to the fp32 output.
  - p_copy = sigmoid(hidden @ Wc + bc) is a [2048,512]x[512,1] matvec,
    computed on the host; the device receives ln(1-p_copy) as an ACT bias
    and a pre-scaled attention (attn * p_copy) for the copy path.
  - Copy path (einsum over src_map) sharded 4 batches per core on the PE,
    emitted before the main loop so it runs while W streams in.
Host side: shard/cast inputs, run SPMD on cores 0-7, normalize + gather.
"""

import numpy as np
import ml_dtypes

bf16 = ml_dtypes.bfloat16

# Problem shape (hardcoded per contract)
B, T, S, C, D, V = 32, 64, 400, 100, 512, 32000
R = B * T              # 2048 rows, row r = t*32 + b
NC = 8
VS = V // NC           # 4000 vocab cols per core
PAD_IDX = 1
NEG_INF = -1e9

KCH = D // 128         # 4 contraction chunks of 128
NRB = R // 128         # 16 row blocks
SCH = 4                # s-chunks of 100 for the copy einsum
OUT_BUFS = 5

_cache = {}


def _build(all_bias: bool):
    import concourse.bass as bass
    import concourse.mybir as mybir
    import concourse.tile as tile
    from concourse import bacc

    fp32 = mybir.dt.float32
    bf = mybir.dt.bfloat16
    AF = mybir.ActivationFunctionType

    nc = bacc.Bacc("TRN2", target_bir_lowering=False, debug=False, num_devices=NC)

    # ---- I/O ----
    hT_d = nc.dram_tensor("hT", [D, R], bf, kind="ExternalInput")
    W_d = nc.dram_tensor("Wk", [D, VS], bf, kind="ExternalInput")
    lnb_d = nc.dram_tensor("lnb", [128, NRB], fp32, kind="ExternalInput")
    attnT_d = nc.dram_tensor("attnT", [S, 256], bf, kind="ExternalInput")
    srcmap_d = nc.dram_tensor("srcmap", [S, 4 * C], bf, kind="ExternalInput")
    out_d = nc.dram_tensor("out", [R, VS], bf, kind="ExternalOutput")
    rs_d = nc.dram_tensor("rs", [128, 8 * NRB], fp32, kind="ExternalOutput")
    cp_d = nc.dram_tensor("cp", [T, 4 * C], fp32, kind="ExternalOutput")
    if all_bias:
        bias_d = nc.dram_tensor("biask", [1, VS], bf, kind="ExternalInput")

    with tile.TileContext(nc) as tc:
        with (
            tc.tile_pool(name="sb", bufs=1) as sb,
            tc.tile_pool(name="ps", bufs=4, space="PSUM") as ps,
        ):
            # ---- resident loads ----
            # sync (HWDGE) ring leads with the eight 0.5MB W column-chunks
            # that gate the matmul stream; the bulk of hT rides at its tail.
            # The gpsimd SWDGE ring carries, in parallel: lnb, the first hT
            # row-chunks (which gate the first stripes), and the copy-path
            # inputs.
            hT_sb = sb.tile([128, KCH, R], bf)
            hT_view = hT_d.ap().rearrange("(c p) r -> p c r", p=128)
            W_sb = sb.tile([128, KCH, VS], bf)
            W_view = W_d.ap().rearrange("(c p) v -> p c v", p=128)
            # sync ring: hT head + all W chunks; gpsimd ring (parallel):
            # lnb, hT row-chunk 1, copy-path inputs, hT bulk.
            nc.sync.dma_start(hT_sb[:, :, 0:512], hT_view[:, :, 0:512])
            for q in range(8):
                nc.sync.dma_start(W_sb[:, :, q * 500:(q + 1) * 500],
                                  W_view[:, :, q * 500:(q + 1) * 500])

            lnb_sb = sb.tile([128, NRB], fp32)
            nc.gpsimd.dma_start(lnb_sb[:, :], lnb_d.ap())
            attnT_sb = sb.tile([100, SCH, 256], bf)
            nc.gpsimd.dma_start(attnT_sb[:, :, :], attnT_d.ap().rearrange("(c p) j -> p c j", p=100))
            srcmap_sb = sb.tile([100, SCH, 4 * C], bf)
            nc.gpsimd.dma_start(srcmap_sb[:, :, :], srcmap_d.ap().rearrange("(c p) j -> p c j", p=100))
            for rq in range(1, 4):
                nc.gpsimd.dma_start(hT_sb[:, :, rq * 512:(rq + 1) * 512],
                                    hT_view[:, :, rq * 512:(rq + 1) * 512])
            if all_bias:
                bias_sb = sb.tile([1, VS], bf)
                nc.gpsimd.dma_start(bias_sb[:, :], bias_d.ap())
                ones_sb = sb.tile([1, 128], bf)
                nc.vector.memset(ones_sb[:, :], 1.0)

            rs_sb = sb.tile([128, 8 * NRB], fp32)  # rowsum partials [p, rb*8+c]
            nc.vector.memset(rs_sb[:, :], 0.0)
            cp_sb = sb.tile([64, 4 * C], fp32)

            ot_tiles = {}

            def get_ot(rb):
                if rb not in ot_tiles:
                    ot_tiles[rb] = sb.tile([128, VS], bf, tag="ot",
                                           bufs=OUT_BUFS, name=f"ot{rb}")
                return ot_tiles[rb]

            def stripe(rb, c0, nb):
                """One nb*500-col stripe: matmuls + exp with fused bias/accum."""
                ot = get_ot(rb)
                st = ps.tile([128, 2, 512], fp32, tag="stripe",
                             name=f"l{rb}_{c0}")
                for kk in range(KCH):
                    for j in range(nb):
                        nc.tensor.matmul(
                            st[:, j, 0:500],
                            hT_sb[:, kk, rb * 128:(rb + 1) * 128],
                            W_sb[:, kk, (c0 + j) * 500:(c0 + j + 1) * 500],
                            start=(kk == 0),
                            stop=(kk == KCH - 1 and not all_bias))
                if all_bias:
                    for j in range(nb):
                        nc.tensor.matmul(
                            st[:, j, 0:500], ones_sb[:, :],
                            bias_sb[:, (c0 + j) * 500:(c0 + j + 1) * 500],
                            start=False, stop=True)
                ev = ot[:, c0 * 500:(c0 + nb) * 500]
                if nb > 1:
                    ev = ev.rearrange("p (g v) -> p g v", g=nb)
                    si = st[:, :, 0:500]
                else:
                    si = st[:, 0, 0:500]
                nc.scalar.activation(
                    ev, si, AF.Exp,
                    bias=lnb_sb[:, rb:rb + 1],
                    accum_out=rs_sb[:, rb * 8 + c0:rb * 8 + c0 + 1])

            def emit_out(rb, c0=0, c1=8):
                nc.sync.dma_start(
                    out_d.ap()[rb * 128:(rb + 1) * 128, c0 * 500:c1 * 500],
                    ot_tiles[rb][:, c0 * 500:c1 * 500])

            # ---- PE warmup: ~3.4us of dummy matmuls on zero tiles so the
            # HAM clock gate is at 8/8 (2.4 GHz) when the real stream
            # starts; they also fill the input-DMA wait ----
            wu_w = sb.tile([128, 128], bf)
            wu_x = sb.tile([128, 512], bf)
            nc.vector.memset(wu_w[:, :], 0.0)
            nc.vector.memset(wu_x[:, :], 0.0)
            wu_ps = ps.tile([128, 2, 512], fp32, tag="stripe", name="warm")
            for i in range(8):
                nc.tensor.matmul(wu_ps[:, 0, :], wu_w[:, :], wu_x[:, :],
                                 start=True, stop=True)

            # ---- phase 1: chunk-major over rb0-3 with 500-col stripes so
            # the PE starts as soon as the first 0.5MB W chunk lands ----
            NW = 4
            for q in range(8):
                for rb in range(NW):
                    stripe(rb, q, 1)
                if q == 3:
                    # copy path: cp[t, bb*C:(bb+1)*C] =
                    #   sum_s attnT[s, bb*64+t] * srcmap[s, bb, :]
                    # (attnT pre-scaled by p_copy on the host)
                    cpps = ps.tile([64, 4 * C], fp32, tag="stripe", name="cpps")
                    for bb in range(4):
                        for c in range(SCH):
                            nc.tensor.matmul(
                                cpps[:, bb * C:(bb + 1) * C],
                                attnT_sb[:, c, bb * 64:(bb + 1) * 64],
                                srcmap_sb[:, c, bb * C:(bb + 1) * C],
                                start=(c == 0), stop=(c == SCH - 1))
                    nc.vector.tensor_copy(cp_sb[:, :], cpps[:, :])
                    nc.gpsimd.dma_start(cp_d.ap(), cp_sb[:, :])
            for rb in range(NW):
                emit_out(rb)

            # ---- phase 2: row-major for the rest, 1000-col stripes; the
            # last row block streams its output in two halves so the final
            # DMA is small ----
            for rb in range(NW, NRB):
                for q in range(4):
                    stripe(rb, 2 * q, 2)
                    if rb == NRB - 1 and q >= 1:
                        # stream the last row block out in shrinking pieces
                        # so the final DMA is small
                        emit_out(rb, 2 * (q - 1) if q == 1 else 2 * q,
                                 2 * q + 2)
                if rb == NRB - 1:
                    # all rowsum slots except rb15's are final now
                    nc.gpsimd.dma_start(rs_d.ap()[:, 0:8 * (NRB - 1)],
                                        rs_sb[:, 0:8 * (NRB - 1)])
                else:
                    emit_out(rb)

            nc.sync.dma_start(rs_d.ap()[:, 8 * (NRB - 1):],
                              rs_sb[:, 8 * (NRB - 1):])

    nc.compile()
    return nc


def _get_nc(all_bias: bool):
    key = ("nc", all_bias)
    if key not in _cache:
        _cache[key] = _build(all_bias)
    return _cache[key]


def kernel(hidden, attn, src_map, W, b, Wc, bc):
    from concourse.bass_utils import run_bass_kernel_spmd

    hidden = np.asarray(hidden, dtype=np.float32)
    attn = np.asarray(attn, dtype=np.float32)
    src_map = np.asarray(src_map, dtype=np.float32)
    W = np.asarray(W, dtype=np.float32)
    b = np.asarray(b, dtype=np.float32)
    Wc = np.asarray(Wc, dtype=np.float32)
    bc = np.asarray(bc, dtype=np.float32)

    all_bias = bool(np.any(b != 0.0))

    # host prologue: p_copy (tiny matvec) and the per-row ACT bias ln(1-p)
    z = hidden.astype(np.float64) @ Wc.astype(np.float64) + bc.astype(np.float64)
    p = 1.0 / (1.0 + np.exp(-z))                         # [R, 1]
    one_m_p = (1.0 - p).reshape(-1)                      # [R]
    lnb = np.log(one_m_p).reshape(NRB, 128).T.astype(np.float32)  # [128, NRB]
    lnb = np.ascontiguousarray(lnb)

    hT = np.ascontiguousarray(hidden.T).astype(bf16)     # [512, 2048]
    attnS = attn * p.astype(np.float32)                  # [R, S] attn * p_copy

    nc = _get_nc(all_bias)

    in_maps = []
    for k in range(NC):
        Wk = np.ascontiguousarray(W[:, k * VS:(k + 1) * VS]).astype(bf16)

        # copy-path shard: batches 4k..4k+3, packed col j = bb*64 + t
        rows = np.array([[t * 32 + 4 * k + bb for t in range(T)] for bb in range(4)])
        rows_flat = rows.reshape(-1)
        attnT_k = np.ascontiguousarray(attnS[rows_flat, :].T).astype(bf16)   # [400, 256]
        srcmap_k = np.ascontiguousarray(
            src_map[:, 4 * k:4 * k + 4, :].reshape(S, 4 * C)).astype(bf16)  # [400, 400]

        im = {"hT": hT, "Wk": Wk, "lnb": lnb, "attnT": attnT_k, "srcmap": srcmap_k}
        if all_bias:
            bias_k = b[k * VS:(k + 1) * VS].astype(np.float64)
            if k == 0:
                bias_k = bias_k.copy()
                bias_k[PAD_IDX] += NEG_INF
            im["biask"] = bias_k.astype(bf16)[None, :]                      # [1, 4000]
        in_maps.append(im)

    global _last_in_maps
    _last_in_maps = in_maps
    res = run_bass_kernel_spmd(nc, in_maps, core_ids=list(range(NC))).results

    # host epilogue: finish the softmax denominator and normalize while
    # upcasting the bf16 shards.
    rs_tot = np.zeros((128, NRB), dtype=np.float64)
    for k in range(NC):
        rsk = res[k]["rs"].astype(np.float64).reshape(128, NRB, 8)
        rs_tot += rsk.sum(axis=2)
    zp = rs_tot.T.reshape(-1)                            # [R] = (1-p) * (Z + e_pad)

    full = np.empty((R, V + C), dtype=np.float32)
    for k in range(NC):
        full[:, k * VS:(k + 1) * VS] = res[k]["out"]

    if all_bias:
        # PAD handled via the -1e9 bias on the device (exp underflows to 0)
        zrow = zp / one_m_p                              # Z_true
    else:
        # device computed exp(0)=1 at the PAD column; remove it from Z
        e_pad = full[:, PAD_IDX].astype(np.float64) / one_m_p
        zrow = zp / one_m_p - e_pad
    scale = (1.0 / zrow).astype(np.float32)
    full[:, :V] *= scale[:, None]
    full[:, PAD_IDX] = 0.0

    t_idx = np.arange(T) * 32
    for k in range(NC):
        cp = res[k]["cp"].reshape(T, 4, C)
        for bb in range(4):
            full[t_idx + 4 * k + bb, V:] = cp[:, bb, :]
    return full
